# revision 55
# baseline (speedup 1.0000x reference)
"""CopyGenerator kernel for 8 TRN2 NeuronCores.

Reference computation (shapes: hidden (50,16,512), attn (50,16,200),
src_map (200,16,20400) one-hot, W (20000,512), b (20000,), Wc (1,512), bc (1,)):

  logits = hidden @ W.T + b            (50,16,20000)
  logits[:, 1, :] = -inf               (masks BATCH index 1)
  prob = softmax(logits, axis=1)       (softmax over the BATCH dim)
  p_copy = sigmoid(hidden @ Wc.T + bc) (50,16,1)
  out_prob = prob * (1 - p_copy)
  copy_prob = einsum('tbs,sbv->tbv', attn * p_copy, src_map)   (50,16,20400)
  copy_prob = copy_prob.reshape(800, 20400).reshape(16, 50, 20400).swapaxes(0,1)
  out = copy_prob ;  out[:, :, :20000] += out_prob

Sharding: tensor-parallel over the extended-vocab dim (2550 cols/core).
The softmax runs over batch (local per (t,v)), and the one-hot scatter only
touches the core's v-slice. Zero communication between cores.

src_map is a one-hot indicator, so the host losslessly converts it to indices
(argmax) and builds, per core and per 128-row v-tile, a COMPACT scatter
matmul: a [K,128] one-hot weight (K = number of source positions whose id
lands in that v-tile, ~20 on average) and a [K,800] block-sparse fp16 rhs
holding attn*p_copy replicated over t at the permuted output columns. One
matmul per (v-tile, psum-half) accumulates the copy path directly onto the
PSUM tile that already holds the softmax result, so no separate merge pass
is needed. K is data-dependent; the compiled program is cached keyed on the
padded K, and a different input pattern simply triggers a recompile (slow but
correct).

On-device layout: v on partitions, (t,b) on the free dim.
  - batch softmax  -> grouped free-dim reduction (groups of 16)
  - +b bias        -> dropped (constant along the softmax dim, cancels)
  - output columns are (b,t)-major: out[v, b*50+t]; the reference's
    reshape/swap permutation makes the copy path's natural (t_o*16+b_o) flat
    index IDENTICAL to the output column index, and the softmax result
    (computed (t,b)-major) is permuted for free through a strided write AP.
  - max-subtraction in softmax is skipped: |logit| < ~8 keeps exp() in range.

Matmuls run in bf16 (logits) / fp16 (copy path); output stored f16.
"""

import sys
import types

sys.path.insert(0, "/opt/trn_rl_repo")

# concourse.bass_utils imports antenv.axon_hooks when trace=True; some
# container images ship an antenv without that module. Inject a stub (and
# register the real NTFF hook if the axon boot shim is present) so tracing
# degrades gracefully instead of crashing.
try:
    import antenv.axon_hooks  # noqa: F401
except Exception:
    try:
        import antenv

        _m = types.ModuleType("antenv.axon_hooks")
        _m._hook = None
        _m.set_axon_ntff_profile_hook = lambda h: setattr(_m, "_hook", h)
        _m.get_axon_ntff_profile_hook = lambda: _m._hook
        sys.modules["antenv.axon_hooks"] = _m
        antenv.axon_hooks = _m
        try:
            from trn_agent_boot.trn_boot import _ntff_profile_via_ctypes

            _m._hook = _ntff_profile_via_ctypes("/opt/axon/libaxon_pjrt.so")
        except Exception:
            pass
    except Exception:
        pass

import numpy as np
import ml_dtypes

import concourse.bass as bass
import concourse.mybir as mybir
from concourse import tile, bacc
from concourse.bass_utils import run_bass_kernel_spmd

BF16 = ml_dtypes.bfloat16

TLEN, BATCH, D = 50, 16, 512
SRC, VOCAB, CVOCAB = 200, 20000, 20400
N_CORES = 8
VC = CVOCAB // N_CORES          # 2550 vocab cols per core
TB = TLEN * BATCH               # 800
PAD_IDX = 1
NVT = (VC + 127) // 128         # 20 v-tiles
P_LAST = VC - (NVT - 1) * 128   # 118
VMASK_PAD = NVT * 128           # 2560
NK = D // 128                   # 4 contraction tiles for d=512
HALVES = ((0, 512), (512, 800))  # psum-bank-aligned column halves
WT_CHUNKS = ((0, 256), (256, 512), (512, 1024), (1024, 1536), (1536, VC))
WT_CHUNK_OF_VT = tuple(
    next(ci for ci, (w0, w1) in enumerate(WT_CHUNKS) if vt * 128 < w1)
    for vt in range(NVT))

_cached = {}


def _build_program(kpad):
    f32 = mybir.dt.float32
    bf = mybir.dt.bfloat16
    f16 = mybir.dt.float16
    kt_rows = [min(128, kpad - k0) for k0 in range(0, kpad, 128)]

    nc = bacc.Bacc("TRN2", target_bir_lowering=False, debug=False,
                   num_devices=N_CORES)

    hid = nc.declare_dram_parameter("hiddenT", [D, TB], bf, isOutput=False)
    wt = nc.declare_dram_parameter("wt", [D, VC], bf, isOutput=False)
    wcp = nc.declare_dram_parameter("wcp", [kpad, NVT * 128], f16,
                                    isOutput=False)
    rcp = nc.declare_dram_parameter("rcp", [kpad, NVT * TB], f16,
                                    isOutput=False)
    omp = nc.declare_dram_parameter("omp", [1, TB], bf, isOutput=False)
    ident = nc.declare_dram_parameter("ident", [128, 128], bf, isOutput=False)
    out = nc.declare_dram_parameter("out", [VC, TB], f16, isOutput=True)

    hid_ap, wt_ap = hid.ap(), wt.ap()
    wcp_ap, rcp_ap, omp_ap, out_ap = wcp.ap(), rcp.ap(), omp.ap(), out.ap()
    ident_ap = ident.ap()

    with tile.TileContext(nc, num_cores=N_CORES) as tc:
        import contextlib

        with contextlib.ExitStack() as ctx:
            const = ctx.enter_context(tc.tile_pool(name="const", bufs=1))
            zp = ctx.enter_context(tc.tile_pool(name="zp", bufs=4))
            sp = ctx.enter_context(tc.tile_pool(name="sp", bufs=4))
            op = ctx.enter_context(tc.tile_pool(name="op", bufs=4))
            ps_a = ctx.enter_context(
                tc.tile_pool(name="ps_a", bufs=2, space="PSUM"))
            ps_b = ctx.enter_context(
                tc.tile_pool(name="ps_b", bufs=2, space="PSUM"))

            # ---- phase 0: load constants ----
            # wt is split into column chunks so the first v-tiles only wait
            # for chunk 0; later chunks stream in behind the pipeline. The
            # four 128-row k-tiles of hid / of each wt chunk are packed
            # side-by-side in one SBUF tile so each load is a single DMA.
            hid_sb = const.tile([128, NK * TB], bf, tag="hid")
            nc.sync.dma_start(
                hid_sb[:].rearrange("p (k c) -> p k c", k=NK),
                hid_ap.rearrange("(k p) c -> p k c", p=128))
            wt_sb = [None] * len(WT_CHUNKS)

            def load_wt_chunk(ci):
                w0, w1 = WT_CHUNKS[ci]
                t = const.tile([128, NK * (w1 - w0)], bf, tag=f"wt{ci}")
                nc.sync.dma_start(
                    t[:].rearrange("p (k c) -> p k c", k=NK),
                    wt_ap[:, w0:w1].rearrange("(k p) c -> p k c", p=128))
                wt_sb[ci] = t

            load_wt_chunk(0)
            omp_row = const.tile([1, TB], bf, tag="omp_row")
            nc.sync.dma_start(omp_row[:], omp_ap[:, :])
            id_sb = const.tile([128, 128], bf, tag="ident")
            nc.sync.dma_start(id_sb[:], ident_ap[:, :])
            wcp_sb, rcp_sb = [], []
            for kt, (k0, nr) in enumerate(
                    (i * 128, r) for i, r in enumerate(kt_rows)):
                tw = const.tile([nr, NVT * 128], f16, tag=f"wcp{kt}")
                nc.sync.dma_start(tw[:], wcp_ap[k0:k0 + nr, :])
                wcp_sb.append(tw)
                tr = const.tile([nr, NVT * TB], f16, tag=f"rcp{kt}")
                nc.sync.dma_start(tr[:], rcp_ap[k0:k0 + nr, :])
                rcp_sb.append(tr)
            for ci in range(1, len(WT_CHUNKS)):
                load_wt_chunk(ci)
            ones_bf = const.tile([1, 128], bf, tag="ones_bf")
            nc.vector.memset(ones_bf[:], 1.0)
            # keep the PE busy while the input DMAs land so HAM has
            # un-throttled the clock before the first real matmul
            warm = const.tile([128, 128], bf, tag="warm")
            nc.vector.memset(warm[:], 0.0)
            wp = ps_a.tile([128, 128], f32, tag="psa")
            for _ in range(40):
                nc.tensor.matmul(wp[:, :], warm[:, :], warm[:, :],
                                 start=True, stop=True)

            # replicate omp across 128 partitions via a ones-vector matmul,
            # stored twice side by side for the pair-fused multiply
            rep_ps = ps_a.tile([128, TB], f32, tag="psa")
            for c0, c1 in HALVES:
                nc.tensor.matmul(rep_ps[:, c0:c1], ones_bf[0:1, :],
                                 omp_row[0:1, c0:c1], start=True, stop=True)
            omp2_sb = const.tile([128, 2 * TB], bf, tag="omp_rep")
            nc.scalar.copy(omp2_sb[:, :TB], rep_ps[:])
            nc.scalar.copy(omp2_sb[:, TB:], rep_ps[:])

            # ---- per v-tile pipeline (pairs share one reciprocal) ----
            def front_half(vt, s_half):
                """logits -> exp -> pad-mask -> reduce -> omp. Returns z."""
                P = 128 if vt < NVT - 1 else P_LAST
                psA = ps_a.tile([128, TB], f32, tag="psa")
                ci = WT_CHUNK_OF_VT[vt]
                wlen = WT_CHUNKS[ci][1] - WT_CHUNKS[ci][0]
                o = vt * 128 - WT_CHUNKS[ci][0]
                for k in range(NK):
                    for c0, c1 in HALVES:
                        nc.tensor.matmul(
                            psA[:P, c0:c1],
                            wt_sb[ci][:, k * wlen + o:k * wlen + o + P],
                            hid_sb[:, k * TB + c0:k * TB + c1],
                            start=(k == 0), stop=(k == NK - 1))
                z = zp.tile([128, TB], bf, tag=f"z{vt % 2}")
                nc.scalar.activation(z[:P, :], psA[:P, :],
                                     mybir.ActivationFunctionType.Exp)
                z3 = z[:P, :].rearrange("p (t b) -> p t b", b=BATCH)
                # batch entry PAD_IDX is -inf-masked in the reference
                nc.gpsimd.memset(z3[:, :, PAD_IDX], 0.0)
                nc.vector.reduce_sum(s_half, z3, axis=mybir.AxisListType.X)
                nc.vector.tensor_mul(z[:P, :], z[:P, :], omp2_sb[:P, :TB])
                return z

            def normalize(vt, z, r_half):
                """zr[v, t*16+b] = z * r (contiguous bf16 write)."""
                P = 128 if vt < NVT - 1 else P_LAST
                z3 = z[:P, :].rearrange("p (t b) -> p t b", b=BATCH)
                zr = zp.tile([128, TB], bf, tag=f"zr{vt % 2}")
                zr3 = zr[:P, :].rearrange("p (t b) -> p t b", b=BATCH)
                r3 = r_half.rearrange("p (t o) -> p t o", o=1)
                z_v, r_b = bass.broadcast_tensor_aps(z3, r3)
                nc.vector.tensor_tensor(zr3, z_v, r_b,
                                        op=mybir.AluOpType.mult)
                return zr

            def back_pe(vt, zr, off):
                """copy matmuls + zr merge -> evict -> store. Emitted one
                pair late so the zr-dependent matmuls never stall logits
                queued behind them on the PE FIFO (HAM stays warm)."""
                P = 128 if vt < NVT - 1 else P_LAST
                v0 = vt * 128
                psB = ps_b.tile([128, TB], f32, tag="psb")
                for kt, nr in enumerate(kt_rows):
                    for c0, c1 in HALVES:
                        nc.tensor.matmul(
                            psB[:, c0:c1],
                            wcp_sb[kt][:, vt * 128:(vt + 1) * 128],
                            rcp_sb[kt][:, vt * TB + c0:vt * TB + c1],
                            start=(kt == 0), stop=False)
                for c0, c1 in HALVES:
                    nc.tensor.matmul(psB[:, c0:c1], id_sb[:, :],
                                     zr[:, off + c0:off + c1],
                                     start=False, stop=True)
                out_sb = op.tile([128, TB], f16, tag="o")
                nc.scalar.copy(out_sb[:P, :], psB[:P, :])
                nc.sync.dma_start(out_ap[v0:v0 + P, :], out_sb[:P, :])

            prev = None
            for pair in range(NVT // 2):
                va, vb = 2 * pair, 2 * pair + 1
                Pa = 128 if va < NVT - 1 else P_LAST
                Pb = 128 if vb < NVT - 1 else P_LAST
                s_t = sp.tile([128, 2 * TLEN], f32, tag="s")
                za = front_half(va, s_t[:Pa, :TLEN])
                zb = front_half(vb, s_t[:Pb, TLEN:])
                r_t = sp.tile([128, 2 * TLEN], f32, tag="r")
                nc.vector.reciprocal_approx_fast(r_t[:, :], s_t[:, :])
                zra = normalize(va, za, r_t[:Pa, :TLEN])
                zrb = normalize(vb, zb, r_t[:Pb, TLEN:])
                if prev is not None:
                    back_pe(prev[0], prev[1], 0)
                    back_pe(prev[2], prev[3], 0)
                prev = (va, zra, vb, zrb)
            back_pe(prev[0], prev[1], 0)
            back_pe(prev[2], prev[3], 0)

    nc.compile()
    return nc


def _prep_inputs(hidden, attn, src_map, W, b, Wc, bc):
    hidden = np.asarray(hidden, dtype=np.float32)
    attn = np.asarray(attn, dtype=np.float32)
    W = np.asarray(W, dtype=np.float32)
    Wc = np.asarray(Wc, dtype=np.float32)
    bc = np.asarray(bc, dtype=np.float32)

    hiddenT = np.ascontiguousarray(
        hidden.reshape(TB, D).T).astype(BF16)          # (512, 800) t-major
    wtp = np.zeros((D, CVOCAB), dtype=BF16)
    wtp[:, :VOCAB] = W.T.astype(BF16)

    # p_copy on host (tiny): sigmoid(hidden @ Wc + bc)
    cl = hidden.reshape(TB, D) @ Wc.reshape(D) + bc.reshape(1)
    pc = 1.0 / (1.0 + np.exp(-cl))                     # (800,) (t,b)-major
    omp_row = (1.0 - pc).astype(BF16).reshape(1, TB)
    pc_tb = pc.reshape(TLEN, BATCH)

    # one-hot src_map -> indices; build per-core compact scatter matmuls
    ids = np.argmax(src_map, axis=2)                   # (200, 16)
    ma = attn * pc_tb[:, :, None]                      # (50, 16, 200)

    core_rows = []
    kmax = 1
    for c in range(N_CORES):
        c0 = c * VC
        s_idx, b_idx = np.nonzero((ids >= c0) & (ids < c0 + VC))
        v = ids[s_idx, b_idx] - c0
        vt = v // 128
        order = np.argsort(vt, kind="stable")
        s_idx, b_idx, v, vt = (s_idx[order], b_idx[order], v[order], vt[order])
        counts = np.bincount(vt, minlength=NVT)
        kmax = max(kmax, int(counts.max()) if len(counts) else 1)
        core_rows.append((s_idx, b_idx, v, vt, counts))

    kpad = -(-kmax // 16) * 16                         # round up to mult of 16
    if kpad > 128:
        kpad = -(-kpad // 128) * 128                   # whole 128-row tiles

    # reference permute: out[t', b'] = copy_orig[f//16, f%16], f = b'*50+t'.
    # Output columns are (t,b)-major (c = t'*16+b'); the copy row for source
    # (s_j, b_j) lands at c(t_o) = (f%50)*16 + f//50 with f = t_o*16 + b_j.
    fvec = np.arange(TLEN) * BATCH
    ident = np.eye(128, dtype=BF16)
    in_maps = []
    for c in range(N_CORES):
        s_idx, b_idx, v, vt, counts = core_rows[c]
        starts = np.concatenate(([0], np.cumsum(counts)))
        wcp = np.zeros((NVT, kpad, 128), dtype=np.float16)
        rcp = np.zeros((NVT, kpad, TB), dtype=np.float16)
        kk = np.arange(len(vt)) - starts[vt]
        wcp[vt, kk, v - vt * 128] = 1.0
        for j in range(len(vt)):
            f = fvec + b_idx[j]
            rcp[vt[j], kk[j], (f % TLEN) * BATCH + f // TLEN] = \
                ma[:, b_idx[j], s_idx[j]]
        sl = slice(c * VC, (c + 1) * VC)
        in_maps.append({
            "hiddenT": hiddenT,
            "wt": np.ascontiguousarray(wtp[:, sl]),
            "wcp": np.ascontiguousarray(
                wcp.transpose(1, 0, 2).reshape(kpad, NVT * 128)),
            "rcp": np.ascontiguousarray(
                rcp.transpose(1, 0, 2).reshape(kpad, NVT * TB)),
            "omp": omp_row,
            "ident": ident,
        })
    # Rows >= VOCAB have all-zero W cols, so the device's softmax path adds
    # bf16(omp)*bf16(1/15) there (15 unmasked batches, exp(0)=1). Reproduce
    # that value exactly and subtract it on the host; b=PAD_IDX columns got
    # z memset to 0 on device, so no correction there.
    r15 = np.float32(BF16(1.0 / 15.0))
    pad_corr = (omp_row.astype(np.float32)[0] * r15).astype(BF16)
    pad_corr = pad_corr.astype(np.float32)
    pad_corr[np.arange(TB) % BATCH == PAD_IDX] = 0.0
    return in_maps, kpad, pad_corr


def kernel(hidden, attn, src_map, W, b, Wc, bc, **run_kwargs):
    in_maps, kpad, pad_corr = _prep_inputs(hidden, attn, src_map, W, b, Wc, bc)
    if kpad not in _cached:
        _cached[kpad] = _build_program(kpad)
    nc = _cached[kpad]
    res = run_bass_kernel_spmd(nc, in_maps, list(range(N_CORES)), **run_kwargs)
    full = np.concatenate([res.results[c]["out"] for c in range(N_CORES)],
                          axis=0).astype(np.float32)   # (20400, 800)
    full[VOCAB:, :] -= pad_corr[None, :]
    out = full.reshape(CVOCAB, TLEN, BATCH).transpose(1, 2, 0)
    if run_kwargs:
        return np.ascontiguousarray(out), res
    return np.ascontiguousarray(out)


# revision 56
# speedup vs baseline: 1.1633x; 1.1633x over previous
"""CopyGenerator kernel for 8 TRN2 NeuronCores.

Reference computation (shapes: hidden (50,16,512), attn (50,16,200),
src_map (200,16,20400) one-hot, W (20000,512), b (20000,), Wc (1,512), bc (1,)):

  logits = hidden @ W.T + b            (50,16,20000)
  logits[:, 1, :] = -inf               (masks BATCH index 1)
  prob = softmax(logits, axis=1)       (softmax over the BATCH dim)
  p_copy = sigmoid(hidden @ Wc.T + bc) (50,16,1)
  out_prob = prob * (1 - p_copy)
  copy_prob = einsum('tbs,sbv->tbv', attn * p_copy, src_map)   (50,16,20400)
  copy_prob = copy_prob.reshape(800, 20400).reshape(16, 50, 20400).swapaxes(0,1)
  out = copy_prob ;  out[:, :, :20000] += out_prob

Sharding: tensor-parallel over the extended-vocab dim (2550 cols/core).
The softmax runs over batch (local per (t,v)), and the one-hot scatter only
touches the core's v-slice. Zero communication between cores.

src_map is a one-hot indicator, so the host losslessly converts it to indices
(argmax) and builds, per core and per 128-row v-tile, a COMPACT scatter
matmul: a [K,128] one-hot weight (K = number of source positions whose id
lands in that v-tile, ~20 on average) and a [K,800] block-sparse fp16 rhs
holding attn*p_copy replicated over t at the permuted output columns. One
matmul per (v-tile, psum-half) accumulates the copy path directly onto the
PSUM tile that already holds the softmax result, so no separate merge pass
is needed. K is data-dependent; the compiled program is cached keyed on the
padded K, and a different input pattern simply triggers a recompile (slow but
correct).

On-device layout: v on partitions, (t,b) on the free dim.
  - batch softmax  -> grouped free-dim reduction (groups of 16)
  - +b bias        -> dropped (constant along the softmax dim, cancels)
  - output columns are (b,t)-major: out[v, b*50+t]; the reference's
    reshape/swap permutation makes the copy path's natural (t_o*16+b_o) flat
    index IDENTICAL to the output column index, and the softmax result
    (computed (t,b)-major) is permuted for free through a strided write AP.
  - max-subtraction in softmax is skipped: |logit| < ~8 keeps exp() in range.

Matmuls run in bf16 (logits) / fp16 (copy path); output stored f16.
"""

import sys
import types

sys.path.insert(0, "/opt/trn_rl_repo")

# concourse.bass_utils imports antenv.axon_hooks when trace=True; some
# container images ship an antenv without that module. Inject a stub (and
# register the real NTFF hook if the axon boot shim is present) so tracing
# degrades gracefully instead of crashing.
try:
    import antenv.axon_hooks  # noqa: F401
except Exception:
    try:
        import antenv

        _m = types.ModuleType("antenv.axon_hooks")
        _m._hook = None
        _m.set_axon_ntff_profile_hook = lambda h: setattr(_m, "_hook", h)
        _m.get_axon_ntff_profile_hook = lambda: _m._hook
        sys.modules["antenv.axon_hooks"] = _m
        antenv.axon_hooks = _m
        try:
            from trn_agent_boot.trn_boot import _ntff_profile_via_ctypes

            _m._hook = _ntff_profile_via_ctypes("/opt/axon/libaxon_pjrt.so")
        except Exception:
            pass
    except Exception:
        pass

import numpy as np
import ml_dtypes

import concourse.bass as bass
import concourse.mybir as mybir
from concourse import tile, bacc
from concourse.bass_utils import run_bass_kernel_spmd

BF16 = ml_dtypes.bfloat16

TLEN, BATCH, D = 50, 16, 512
SRC, VOCAB, CVOCAB = 200, 20000, 20400
N_CORES = 8
VC = CVOCAB // N_CORES          # 2550 vocab cols per core
TB = TLEN * BATCH               # 800
PAD_IDX = 1
NVT = (VC + 127) // 128         # 20 v-tiles
P_LAST = VC - (NVT - 1) * 128   # 118
VMASK_PAD = NVT * 128           # 2560
NK = D // 128                   # 4 contraction tiles for d=512
HALVES = ((0, 512), (512, 800))  # psum-bank-aligned column halves
WT_CHUNKS = ((0, 256), (256, 512), (512, 1024), (1024, 1536), (1536, VC))
WT_CHUNK_OF_VT = tuple(
    next(ci for ci, (w0, w1) in enumerate(WT_CHUNKS) if vt * 128 < w1)
    for vt in range(NVT))

_cached = {}


def _build_program(kpad):
    f32 = mybir.dt.float32
    bf = mybir.dt.bfloat16
    f16 = mybir.dt.float16
    kt_rows = [min(128, kpad - k0) for k0 in range(0, kpad, 128)]

    nc = bacc.Bacc("TRN2", target_bir_lowering=False, debug=False,
                   num_devices=N_CORES)

    hid = nc.declare_dram_parameter("hiddenT", [D, TB], bf, isOutput=False)
    wt = nc.declare_dram_parameter("wt", [D, VC], bf, isOutput=False)
    wcp = nc.declare_dram_parameter("wcp", [kpad, NVT * 128], f16,
                                    isOutput=False)
    rcp = nc.declare_dram_parameter("rcp", [kpad, NVT * TB], f16,
                                    isOutput=False)
    omp = nc.declare_dram_parameter("omp", [1, TB], bf, isOutput=False)
    ident = nc.declare_dram_parameter("ident", [128, 128], bf, isOutput=False)
    out = nc.declare_dram_parameter("out", [VC, TB], f16, isOutput=True)

    hid_ap, wt_ap = hid.ap(), wt.ap()
    wcp_ap, rcp_ap, omp_ap, out_ap = wcp.ap(), rcp.ap(), omp.ap(), out.ap()
    ident_ap = ident.ap()

    with tile.TileContext(nc, num_cores=N_CORES) as tc:
        import contextlib

        with contextlib.ExitStack() as ctx:
            const = ctx.enter_context(tc.tile_pool(name="const", bufs=1))
            zp = ctx.enter_context(tc.tile_pool(name="zp", bufs=3))
            sp = ctx.enter_context(tc.tile_pool(name="sp", bufs=3))
            op = ctx.enter_context(tc.tile_pool(name="op", bufs=3))
            ps_a = ctx.enter_context(
                tc.tile_pool(name="ps_a", bufs=2, space="PSUM"))
            ps_b = ctx.enter_context(
                tc.tile_pool(name="ps_b", bufs=2, space="PSUM"))

            # ---- phase 0: load constants ----
            # wt is split into column chunks so the first v-tiles only wait
            # for chunk 0; later chunks stream in behind the pipeline. The
            # four 128-row k-tiles of hid / of each wt chunk are packed
            # side-by-side in one SBUF tile so each load is a single DMA.
            hid_sb = const.tile([128, NK * TB], bf, tag="hid")
            nc.sync.dma_start(
                hid_sb[:].rearrange("p (k c) -> p k c", k=NK),
                hid_ap.rearrange("(k p) c -> p k c", p=128))
            wt_sb = [None] * len(WT_CHUNKS)

            def load_wt_chunk(ci):
                w0, w1 = WT_CHUNKS[ci]
                t = const.tile([128, NK * (w1 - w0)], bf, tag=f"wt{ci}")
                nc.sync.dma_start(
                    t[:].rearrange("p (k c) -> p k c", k=NK),
                    wt_ap[:, w0:w1].rearrange("(k p) c -> p k c", p=128))
                wt_sb[ci] = t

            load_wt_chunk(0)
            omp_row = const.tile([1, TB], bf, tag="omp_row")
            nc.sync.dma_start(omp_row[:], omp_ap[:, :])
            id_sb = const.tile([128, 128], bf, tag="ident")
            nc.sync.dma_start(id_sb[:], ident_ap[:, :])
            wcp_sb, rcp_sb = [], []
            for kt, (k0, nr) in enumerate(
                    (i * 128, r) for i, r in enumerate(kt_rows)):
                tw = const.tile([nr, NVT * 128], f16, tag=f"wcp{kt}")
                nc.sync.dma_start(tw[:], wcp_ap[k0:k0 + nr, :])
                wcp_sb.append(tw)
                tr = const.tile([nr, NVT * TB], f16, tag=f"rcp{kt}")
                nc.sync.dma_start(tr[:], rcp_ap[k0:k0 + nr, :])
                rcp_sb.append(tr)
            for ci in range(1, len(WT_CHUNKS)):
                load_wt_chunk(ci)
            ones_bf = const.tile([1, 128], bf, tag="ones_bf")
            nc.vector.memset(ones_bf[:], 1.0)
            # keep the PE busy while the input DMAs land so HAM has
            # un-throttled the clock before the first real matmul
            warm = const.tile([128, 128], bf, tag="warm")
            nc.vector.memset(warm[:], 0.0)
            wp = ps_a.tile([128, 128], f32, tag="psa")
            for _ in range(40):
                nc.tensor.matmul(wp[:, :], warm[:, :], warm[:, :],
                                 start=True, stop=True)

            # replicate omp across 128 partitions via a ones-vector matmul,
            # stored twice side by side for the pair-fused multiply
            rep_ps = ps_a.tile([128, TB], f32, tag="psa")
            for c0, c1 in HALVES:
                nc.tensor.matmul(rep_ps[:, c0:c1], ones_bf[0:1, :],
                                 omp_row[0:1, c0:c1], start=True, stop=True)
            omp2_sb = const.tile([128, 2 * TB], bf, tag="omp_rep")
            nc.scalar.copy(omp2_sb[:, :TB], rep_ps[:])
            nc.scalar.copy(omp2_sb[:, TB:], rep_ps[:])

            # ---- per v-tile pipeline (pairs share one reciprocal) ----
            def front_half(vt, s_half):
                """logits -> exp -> pad-mask -> reduce -> omp. Returns z."""
                P = 128 if vt < NVT - 1 else P_LAST
                psA = ps_a.tile([128, TB], f32, tag="psa")
                ci = WT_CHUNK_OF_VT[vt]
                wlen = WT_CHUNKS[ci][1] - WT_CHUNKS[ci][0]
                o = vt * 128 - WT_CHUNKS[ci][0]
                for k in range(NK):
                    for c0, c1 in HALVES:
                        nc.tensor.matmul(
                            psA[:P, c0:c1],
                            wt_sb[ci][:, k * wlen + o:k * wlen + o + P],
                            hid_sb[:, k * TB + c0:k * TB + c1],
                            start=(k == 0), stop=(k == NK - 1))
                z = zp.tile([128, TB], bf, tag=f"z{vt % 2}")
                nc.scalar.activation(z[:P, :], psA[:P, :],
                                     mybir.ActivationFunctionType.Exp)
                z3 = z[:P, :].rearrange("p (t b) -> p t b", b=BATCH)
                # batch entry PAD_IDX is -inf-masked in the reference
                nc.gpsimd.memset(z3[:, :, PAD_IDX], 0.0)
                nc.vector.reduce_sum(s_half, z3, axis=mybir.AxisListType.X)
                nc.vector.tensor_mul(z[:P, :], z[:P, :], omp2_sb[:P, :TB])
                return z

            def normalize(vt, z, r_half):
                """zr[v, t*16+b] = z * r (contiguous bf16 write)."""
                P = 128 if vt < NVT - 1 else P_LAST
                z3 = z[:P, :].rearrange("p (t b) -> p t b", b=BATCH)
                zr = zp.tile([128, TB], bf, tag=f"zr{vt % 2}")
                zr3 = zr[:P, :].rearrange("p (t b) -> p t b", b=BATCH)
                r3 = r_half.rearrange("p (t o) -> p t o", o=1)
                z_v, r_b = bass.broadcast_tensor_aps(z3, r3)
                nc.vector.tensor_tensor(zr3, z_v, r_b,
                                        op=mybir.AluOpType.mult)
                return zr

            def back_pe(vt, zr, off):
                """copy matmuls + zr merge -> evict -> store. Emitted one
                pair late so the zr-dependent matmuls never stall logits
                queued behind them on the PE FIFO (HAM stays warm)."""
                P = 128 if vt < NVT - 1 else P_LAST
                v0 = vt * 128
                psB = ps_b.tile([128, TB], f32, tag="psb")
                for kt, nr in enumerate(kt_rows):
                    for c0, c1 in HALVES:
                        nc.tensor.matmul(
                            psB[:, c0:c1],
                            wcp_sb[kt][:, vt * 128:(vt + 1) * 128],
                            rcp_sb[kt][:, vt * TB + c0:vt * TB + c1],
                            start=(kt == 0), stop=False)
                for c0, c1 in HALVES:
                    nc.tensor.matmul(psB[:, c0:c1], id_sb[:, :],
                                     zr[:, off + c0:off + c1],
                                     start=False, stop=True)
                out_sb = op.tile([128, TB], f16, tag="o")
                nc.scalar.copy(out_sb[:P, :], psB[:P, :])
                nc.sync.dma_start(out_ap[v0:v0 + P, :], out_sb[:P, :])

            prev = None
            for pair in range(NVT // 2):
                va, vb = 2 * pair, 2 * pair + 1
                Pa = 128 if va < NVT - 1 else P_LAST
                Pb = 128 if vb < NVT - 1 else P_LAST
                s_t = sp.tile([128, 2 * TLEN], f32, tag="s")
                za = front_half(va, s_t[:Pa, :TLEN])
                zb = front_half(vb, s_t[:Pb, TLEN:])
                r_t = sp.tile([128, 2 * TLEN], f32, tag="r")
                nc.vector.reciprocal_approx_fast(r_t[:, :], s_t[:, :])
                zra = normalize(va, za, r_t[:Pa, :TLEN])
                zrb = normalize(vb, zb, r_t[:Pb, TLEN:])
                if prev is not None:
                    back_pe(prev[0], prev[1], 0)
                    back_pe(prev[2], prev[3], 0)
                prev = (va, zra, vb, zrb)
            back_pe(prev[0], prev[1], 0)
            back_pe(prev[2], prev[3], 0)

    nc.compile()
    return nc


def _prep_inputs(hidden, attn, src_map, W, b, Wc, bc):
    hidden = np.asarray(hidden, dtype=np.float32)
    attn = np.asarray(attn, dtype=np.float32)
    W = np.asarray(W, dtype=np.float32)
    Wc = np.asarray(Wc, dtype=np.float32)
    bc = np.asarray(bc, dtype=np.float32)

    hiddenT = np.ascontiguousarray(
        hidden.reshape(TB, D).T).astype(BF16)          # (512, 800) t-major
    wtp = np.zeros((D, CVOCAB), dtype=BF16)
    wtp[:, :VOCAB] = W.T.astype(BF16)

    # p_copy on host (tiny): sigmoid(hidden @ Wc + bc)
    cl = hidden.reshape(TB, D) @ Wc.reshape(D) + bc.reshape(1)
    pc = 1.0 / (1.0 + np.exp(-cl))                     # (800,) (t,b)-major
    omp_row = (1.0 - pc).astype(BF16).reshape(1, TB)
    pc_tb = pc.reshape(TLEN, BATCH)

    # one-hot src_map -> indices; build per-core compact scatter matmuls
    ids = np.argmax(src_map, axis=2)                   # (200, 16)
    ma = attn * pc_tb[:, :, None]                      # (50, 16, 200)

    core_rows = []
    kmax = 1
    for c in range(N_CORES):
        c0 = c * VC
        s_idx, b_idx = np.nonzero((ids >= c0) & (ids < c0 + VC))
        v = ids[s_idx, b_idx] - c0
        vt = v // 128
        order = np.argsort(vt, kind="stable")
        s_idx, b_idx, v, vt = (s_idx[order], b_idx[order], v[order], vt[order])
        counts = np.bincount(vt, minlength=NVT)
        kmax = max(kmax, int(counts.max()) if len(counts) else 1)
        core_rows.append((s_idx, b_idx, v, vt, counts))

    kpad = -(-kmax // 16) * 16                         # round up to mult of 16
    if kpad > 128:
        kpad = -(-kpad // 128) * 128                   # whole 128-row tiles

    # reference permute: out[t', b'] = copy_orig[f//16, f%16], f = b'*50+t'.
    # Output columns are (t,b)-major (c = t'*16+b'); the copy row for source
    # (s_j, b_j) lands at c(t_o) = (f%50)*16 + f//50 with f = t_o*16 + b_j.
    fvec = np.arange(TLEN) * BATCH
    ident = np.eye(128, dtype=BF16)
    in_maps = []
    for c in range(N_CORES):
        s_idx, b_idx, v, vt, counts = core_rows[c]
        starts = np.concatenate(([0], np.cumsum(counts)))
        wcp = np.zeros((NVT, kpad, 128), dtype=np.float16)
        rcp = np.zeros((NVT, kpad, TB), dtype=np.float16)
        kk = np.arange(len(vt)) - starts[vt]
        wcp[vt, kk, v - vt * 128] = 1.0
        for j in range(len(vt)):
            f = fvec + b_idx[j]
            rcp[vt[j], kk[j], (f % TLEN) * BATCH + f // TLEN] = \
                ma[:, b_idx[j], s_idx[j]]
        sl = slice(c * VC, (c + 1) * VC)
        in_maps.append({
            "hiddenT": hiddenT,
            "wt": np.ascontiguousarray(wtp[:, sl]),
            "wcp": np.ascontiguousarray(
                wcp.transpose(1, 0, 2).reshape(kpad, NVT * 128)),
            "rcp": np.ascontiguousarray(
                rcp.transpose(1, 0, 2).reshape(kpad, NVT * TB)),
            "omp": omp_row,
            "ident": ident,
        })
    # Rows >= VOCAB have all-zero W cols, so the device's softmax path adds
    # bf16(omp)*bf16(1/15) there (15 unmasked batches, exp(0)=1). Reproduce
    # that value exactly and subtract it on the host; b=PAD_IDX columns got
    # z memset to 0 on device, so no correction there.
    r15 = np.float32(BF16(1.0 / 15.0))
    pad_corr = (omp_row.astype(np.float32)[0] * r15).astype(BF16)
    pad_corr = pad_corr.astype(np.float32)
    pad_corr[np.arange(TB) % BATCH == PAD_IDX] = 0.0
    return in_maps, kpad, pad_corr


def kernel(hidden, attn, src_map, W, b, Wc, bc, **run_kwargs):
    in_maps, kpad, pad_corr = _prep_inputs(hidden, attn, src_map, W, b, Wc, bc)
    if kpad not in _cached:
        _cached[kpad] = _build_program(kpad)
    nc = _cached[kpad]
    res = run_bass_kernel_spmd(nc, in_maps, list(range(N_CORES)), **run_kwargs)
    full = np.concatenate([res.results[c]["out"] for c in range(N_CORES)],
                          axis=0).astype(np.float32)   # (20400, 800)
    full[VOCAB:, :] -= pad_corr[None, :]
    out = full.reshape(CVOCAB, TLEN, BATCH).transpose(1, 2, 0)
    if run_kwargs:
        return np.ascontiguousarray(out), res
    return np.ascontiguousarray(out)
